# revision 20
# baseline (speedup 1.0000x reference)
"""Trainium2 Bass kernel for nn_ClusterForecasting (vq_codebook).

Model: enc-linear -> 2x pre-LN transformer encoder layers -> nearest-centroid
VQ loss + output sequence.  Data-parallel over batch: 8 items per NeuronCore,
weights replicated.  Activations in float32r (TF32-class PE throughput),
residual stream in fp32.

Key layout choices (per item):
  - residual stream h_tm:   [128 part=t%128, 4 t-chunks, 512 d]  (token-major)
  - h_lnT (for matmul lhsT): [128 part=d%128, 4 d-subs, 512 t]   (feature-major)
    built via normal matmuls against the identity (PE transpose-mode carries
    only one HW wait slot and fails codegen on joins).
  - Q,K produced feature-major, V token-major, scores computed transposed
    (scoresT[kt, q]), softmax denominators via masked-ones matmuls,
    normalization folded into the PSUM->SBUF copy of attention output.
  - VQ loss needs no argmin/gather: loss = sum_t [ |o_t|^2 + min_c(|c|^2 - 2 o_t.c) ] / (B*D)
"""
import sys
sys.path.insert(0, '/opt/trn_rl_repo')

import numpy as np
from contextlib import ExitStack

import concourse.bass as bass
import concourse.mybir as mybir
import concourse.tile as tile
from concourse import bass_utils

F32 = mybir.dt.float32
F32R = mybir.dt.float32r
AF = mybir.ActivationFunctionType
OP = mybir.AluOpType
AX = mybir.AxisListType

B, S, F, D, H, L, K = 64, 512, 8, 512, 8, 2, 512
FF = 4 * D
DH = D // H
NCORES = 8
IPC = B // NCORES          # items per core
P = 128
TC = S // P                # 4 token chunks
DS = D // P                # 4 feature subs
FS = FF // P               # 16 ff subs


def split_waits(nc, limit=1):
    """Walrus codegen allows a single sync-wait slot per instruction; move
    excess waits onto injected NOPs on the same engine."""
    n = 0
    for fn in nc.m.functions:
        for bb in fn.blocks:
            out = []
            for inst in bb.instructions:
                si = getattr(inst, 'sync_info', None)
                ow = list(si.on_wait) if (si is not None and si.on_wait) else []
                eng = getattr(inst, 'engine', None)
                if len(ow) > limit and eng is not None and eng != mybir.EngineType.Unassigned:
                    extra, keep = ow[:-limit], ow[-limit:]
                    for w in extra:
                        nop = mybir.InstNoOp(
                            name=nc.get_next_instruction_name(),
                            ins=[], outs=[], engine=eng)
                        nop.sync_info = mybir.SyncInfo(on_wait=[w], on_update=[])
                        out.append(nop)
                        n += 1
                    inst.sync_info = mybir.SyncInfo(
                        on_wait=keep, on_update=list(si.on_update or []))
                out.append(inst)
            bb.instructions = out
    return n


def build_bass():
    nc = bass.Bass("TRN2", target_bir_lowering=False, debug=False,
                   num_devices=NCORES)

    def din(name, shape, dt=F32R):
        return nc.dram_tensor(name, shape, dt, kind="ExternalInput").ap()

    xT_d = din("xT", [IPC, F, S])                       # x transposed per item
    encW_d = din("encW", [F, D])
    encb_d = din("encb", [P, D], F32)                   # broadcast rows
    wqkv_d = din("wqkv", [L, P, DS, 3 * D])
    wo_d = din("wo", [L, P, DS, D])
    w1_d = din("w1", [L, P, DS, FF])
    w2_d = din("w2", [L, P, FS, D])
    b1_d = din("b1", [L, P, FS], F32)
    b2_d = din("b2", [L, P, D], F32)                    # broadcast rows
    ctrT_d = din("ctrT", [P, DS, K])                    # centers^T rearranged
    cn2_d = din("cn2", [P, K], F32)                     # broadcast rows
    identr_d = din("identr", [P, P])
    ident32_d = din("ident32", [P, P], F32)
    sel_d = din("sel", [64, P])                         # denom bcast selector
    ones_d = din("ones", [P, 1], F32)

    out_d = nc.dram_tensor("out_seq", [IPC, S, D], F32, kind="ExternalOutput").ap()
    loss_d = nc.dram_tensor("loss_part", [1, 1], F32, kind="ExternalOutput").ap()

    with tile.TileContext(nc) as tc, ExitStack() as ctx:
        singles = ctx.enter_context(tc.tile_pool(name="singles", bufs=1))
        wpool = ctx.enter_context(tc.tile_pool(name="wpool", bufs=1))
        acts = ctx.enter_context(tc.tile_pool(name="acts", bufs=1))
        acts2 = ctx.enter_context(tc.tile_pool(name="acts2", bufs=2))
        small = ctx.enter_context(tc.tile_pool(name="small", bufs=4))
        dram = ctx.enter_context(tc.tile_pool(name="dram", bufs=1, space="DRAM"))
        psP = ctx.enter_context(tc.tile_pool(name="psP", bufs=4, space="PSUM"))
        psS = ctx.enter_context(tc.tile_pool(name="psS", bufs=2, space="PSUM"))
        psV = ctx.enter_context(tc.tile_pool(name="psV", bufs=2, space="PSUM"))

        # ---- constants (resident both phases) ----
        identr = singles.tile([P, P], F32R)
        nc.sync.dma_start(identr[:], identr_d)
        ident32 = singles.tile([P, P], F32)
        nc.sync.dma_start(ident32[:], ident32_d)
        sel = singles.tile([64, P], F32R)
        nc.sync.dma_start(sel[:], sel_d)
        den2 = singles.tile([64, S], F32R)
        encb = singles.tile([P, D], F32)
        nc.sync.dma_start(encb[:], encb_d)
        encW = singles.tile([F, D], F32R)
        nc.sync.dma_start(encW[:], encW_d)
        ctrT = singles.tile([P, DS, K], F32R)
        nc.sync.dma_start(ctrT[:], ctrT_d)
        cn2 = singles.tile([P, K], F32)
        nc.sync.dma_start(cn2[:], cn2_d)
        ones_c = singles.tile([P, 1], F32)
        nc.sync.dma_start(ones_c[:], ones_d)
        loss_cols = singles.tile([P, 1], F32)
        nc.vector.memset(loss_cols[:], 0.0)
        eps_t = singles.tile([P, 1], F32)
        nc.vector.memset(eps_t[:], 1e-5)
        zero_t = singles.tile([P, 1], F32)
        nc.vector.memset(zero_t[:], 0.0)
        nc.vector.tensor_copy(den2[:], zero_t[0:64, 0:1].to_broadcast((64, S)))

        h1_dram = dram.tile([IPC, P, TC, D], F32)

        def load_layer(l):
            wqkv = wpool.tile([P, DS, 3 * D], F32R, tag="wqkv")
            nc.sync.dma_start(wqkv[:], wqkv_d[l])
            wo = wpool.tile([P, DS, D], F32R, tag="wo")
            nc.sync.dma_start(wo[:], wo_d[l])
            w1 = wpool.tile([P, DS, FF], F32R, tag="w1")
            nc.sync.dma_start(w1[:], w1_d[l])
            w2 = wpool.tile([P, FS, D], F32R, tag="w2")
            nc.sync.dma_start(w2[:], w2_d[l])
            b1 = wpool.tile([P, FS], F32, tag="b1")
            nc.sync.dma_start(b1[:], b1_d[l])
            b2 = wpool.tile([P, D], F32, tag="b2")
            nc.sync.dma_start(b2[:], b2_d[l])
            return wqkv, wo, w1, w2, b1, b2

        def layer_norm(h_tm, tag):
            """token-major LN: returns h_ln (f32r) [P, TC, D]"""
            h_ln = acts.tile([P, TC, D], F32R, tag=tag)
            for t in range(TC):
                st = small.tile([P, 6], F32, tag="bnst")
                nc.vector.bn_stats(st[:], h_tm[:, t, :])
                mv = small.tile([P, 2], F32, tag="bnmv")
                nc.vector.bn_aggr(mv[:], st[:])
                sd = small.tile([P, 1], F32, tag="sd")
                nc.scalar.activation(sd[:], mv[:, 1:2], AF.Sqrt, bias=eps_t[:], scale=1.0)
                rs = small.tile([P, 1], F32, tag="rs")
                nc.vector.reciprocal(rs[:], sd[:])
                nm = small.tile([P, 1], F32, tag="nm")
                nc.vector.scalar_tensor_tensor(
                    nm[:], mv[:, 0:1], -1.0, rs[:], op0=OP.mult, op1=OP.mult)
                nc.scalar.activation(h_ln[:, t, :], h_tm[:, t, :], AF.Identity,
                                     bias=nm[:], scale=rs[:])
            return h_ln

        def transpose4(src, s_cols, dtag, in_f32=False):
            """transpose [S tokens, 128 d-cols starting s_cols] -> hT [P, TC*P]
            via 4 normal matmuls against identity; src is [P, TC, D]-style."""
            ps = psP.tile([P, S], F32, tag="ps")
            ident = ident32 if in_f32 else identr
            for t in range(TC):
                nc.tensor.matmul(ps[:, t * P:(t + 1) * P],
                                 src[:, t, s_cols:s_cols + P], ident[:],
                                 start=True, stop=True)
            dst = acts.tile([P, DS, S], F32R, tag=dtag)
            return ps, dst

        def build_hT(src, dtag, in_f32=False):
            """full transpose: src [P, TC, D] -> dst [P, DS, S] (f32r)
            via PE transpose-mode (split_waits handles its 1-wait ISA slot)"""
            dst = None
            for s in range(DS):
                ps = psP.tile([P, S], F32 if in_f32 else F32R, tag="ps")
                ident = ident32 if in_f32 else identr
                for t in range(TC):
                    nc.tensor.transpose(ps[:, t * P:(t + 1) * P],
                                        src[:, t, s * P:(s + 1) * P], ident[:])
                if dst is None:
                    dst = acts.tile([P, DS, S], F32R, tag=dtag)
                nc.vector.tensor_copy(dst[:, s, :], ps[:])
            return dst

        def emit_layer(h_tm, wqkv, wo, w1, w2, b1, b2):
            # ---- LN1 + transpose ----
            h_ln = layer_norm(h_tm, "h_ln")
            hT = build_hT(h_ln, "hT")
            # ---- QKV ----
            q_fm = acts.tile([P, DS, S], F32R, tag="q_fm")
            k_fm = acts.tile([P, DS, S], F32R, tag="k_fm")
            for c in range(8):
                ps = psP.tile([P, S], F32, tag="ps")
                for s in range(DS):
                    nc.tensor.matmul(ps[:], wqkv[:, s, c * P:(c + 1) * P],
                                     hT[:, s, :], start=(s == 0), stop=(s == DS - 1))
                dst = q_fm[:, c, :] if c < 4 else k_fm[:, c - 4, :]
                nc.vector.tensor_copy(dst, ps[:])
            v_ext = acts.tile([P, TC, H, 65], F32R, tag="v_tm")
            nc.vector.tensor_copy(v_ext[:, :, :, 64:65],
                                  ones_c[:, None, None, :].to_broadcast((P, TC, H, 1)))
            for t in range(TC):
                ps = psP.tile([P, S], F32, tag="ps")
                for s in range(DS):
                    nc.tensor.matmul(ps[:], hT[:, s, t * P:(t + 1) * P],
                                     wqkv[:, s, 2 * D:3 * D],
                                     start=(s == 0), stop=(s == DS - 1))
                nc.vector.tensor_copy(
                    v_ext[:, t, :, 0:64],
                    ps[:].rearrange("p (h dv) -> p h dv", h=H))
            # ---- attention: scores of a head pair interleaved (disjoint 64-row
            # groups run concurrently on the PE); softmax denominator comes for
            # free from the ones column of v_ext (PV psum row 64) ----
            o_fm = acts.tile([P, DS, S], F32R, tag="o_fm")
            for pair in range(4):
                expTa = acts.tile([P, TC, S], F32R, tag="expTa")
                expTb = acts.tile([P, TC, S], F32R, tag="expTb")
                expTs = [expTa, expTb]
                for kc in range(TC):
                    for hh in range(2):
                        r0 = hh * 64
                        sc = psS.tile([P, S], F32)
                        nc.tensor.matmul(sc[:],
                                         k_fm[r0:r0 + 64, pair, kc * P:(kc + 1) * P],
                                         q_fm[r0:r0 + 64, pair, :],
                                         start=True, stop=True,
                                         tile_position=(r0, 0))
                        nc.scalar.activation(expTs[hh][:, kc, :], sc[:], AF.Exp,
                                             bias=zero_t[:], scale=float(1.0 / np.sqrt(DH)))
                pvs = []
                for hh in range(2):
                    h = 2 * pair + hh
                    pv = psV.tile([65, S], F32)
                    for kc in range(TC):
                        nc.tensor.matmul(pv[:], v_ext[:, kc, h, :],
                                         expTs[hh][:, kc, :],
                                         start=(kc == 0), stop=(kc == TC - 1))
                    nc.vector.tensor_copy(den2[hh * 32:hh * 32 + 1, :], pv[64:65, :])
                    pvs.append(pv)
                den_ps = psP.tile([P, S], F32, tag="ps")
                nc.tensor.matmul(den_ps[:], sel[:], den2[:], start=True, stop=True)
                recip = acts.tile([P, S], F32, tag="recip")
                nc.vector.reciprocal(recip[:], den_ps[:])
                for hh in range(2):
                    nc.vector.tensor_mul(o_fm[hh * 64:hh * 64 + 64, pair, :],
                                         pvs[hh][0:64, :], recip[hh * 64:hh * 64 + 64, :])
            # ---- Wo + residual ----
            for t in range(TC):
                ps = psP.tile([P, S], F32, tag="ps")
                for s in range(DS):
                    nc.tensor.matmul(ps[:], o_fm[:, s, t * P:(t + 1) * P],
                                     wo[:, s, :], start=(s == 0), stop=(s == DS - 1))
                nc.vector.tensor_add(h_tm[:, t, :], h_tm[:, t, :], ps[:])
            # ---- LN2 + transpose ----
            h_ln2 = layer_norm(h_tm, "h_ln")
            hT2 = build_hT(h_ln2, "hT")
            # ---- FFN: FFN2 accumulates across all 16 ff-subtiles in four
            # persistent PSUM banks (borrowed from the idle attention pools) ----
            for t in range(TC):
                nc.vector.tensor_add(h_tm[:, t, :], h_tm[:, t, :], b2[:])
            f2 = []
            for _fi in range(4):
                _fp = psS if _fi < 2 else psV
                _ft = "sc" if _fi < 2 else "pv"
                f2t = _fp.tile([P, S], F32, tag=_ft, name=f"f2_{_fi}")
                f2.append(f2t)
            for fc in range(8):
                a_ch = acts.tile([P, 2, S], F32R, tag="a_ch")
                for fs in range(2):
                    ffi = fc * 2 + fs
                    ps = psP.tile([P, S], F32, tag="ps")
                    for s in range(DS):
                        nc.tensor.matmul(ps[:], w1[:, s, ffi * P:(ffi + 1) * P],
                                         hT2[:, s, :], start=(s == 0), stop=(s == DS - 1))
                    nc.scalar.activation(a_ch[:, fs, :], ps[:], AF.Relu,
                                         bias=b1[:, ffi:ffi + 1], scale=1.0)
                for t in range(TC):
                    for fs in range(2):
                        nc.tensor.matmul(f2[t][:], a_ch[:, fs, t * P:(t + 1) * P],
                                         w2[:, fc * 2 + fs, :],
                                         start=(fc == 0 and fs == 0),
                                         stop=(fc == 7 and fs == 1),
                                         skip_group_check=True)
            for t in range(TC):
                nc.vector.tensor_add(h_tm[:, t, :], h_tm[:, t, :], f2[t][:])

        # ================= phase A: enc + layer 0 =================
        wqkv, wo, w1, w2, b1, b2 = load_layer(0)
        for it in range(IPC):
            xT = acts.tile([F, S], F32R, tag="xT")
            nc.sync.dma_start(xT[:], xT_d[it])
            h_tm = acts2.tile([P, TC, D], F32, tag="h_tm")
            for t in range(TC):
                ps = psP.tile([P, S], F32, tag="ps")
                nc.tensor.matmul(ps[:], xT[:, t * P:(t + 1) * P], encW[:],
                                 start=True, stop=True)
                nc.vector.tensor_add(h_tm[:, t, :], ps[:], encb[:])
            emit_layer(h_tm, wqkv, wo, w1, w2, b1, b2)
            nc.sync.dma_start(h1_dram[it], h_tm[:])

        # ================= phase B: layer 1 + VQ =================
        wqkv, wo, w1, w2, b1, b2 = load_layer(1)
        for it in range(IPC):
            h_tm = acts2.tile([P, TC, D], F32, tag="h_tm")
            nc.sync.dma_start(h_tm[:], h1_dram[it])
            emit_layer(h_tm, wqkv, wo, w1, w2, b1, b2)
            # output
            nc.sync.dma_start(
                out_d[it].rearrange("(t p) d -> p t d", p=P), h_tm[:])
            # VQ distances
            hT = build_hT(h_tm, "hT", in_f32=True)
            for t in range(TC):
                ps = psP.tile([P, S], F32, tag="ps")
                for s in range(DS):
                    nc.tensor.matmul(ps[:], hT[:, s, t * P:(t + 1) * P],
                                     ctrT[:, s, :], start=(s == 0), stop=(s == DS - 1))
                tmp = acts.tile([P, K], F32, tag="dtmp")
                nc.vector.scalar_tensor_tensor(
                    tmp[:], ps[:], -2.0, cn2[:], op0=OP.mult, op1=OP.add)
                mn = small.tile([P, 1], F32, tag="mn")
                nc.vector.tensor_reduce(mn[:], tmp[:], axis=AX.X, op=OP.min)
                sq = acts.tile([P, D], F32, tag="dtmp")
                on2 = small.tile([P, 1], F32, tag="on2")
                nc.scalar.activation(sq[:], h_tm[:, t, :], AF.Square,
                                     bias=zero_t[:], accum_out=on2[:])
                nc.vector.tensor_add(loss_cols[:], loss_cols[:], mn[:])
                nc.vector.tensor_add(loss_cols[:], loss_cols[:], on2[:])

        lps = psP.tile([1, 1], F32, tag="ps")
        nc.tensor.matmul(lps[:], ones_c[:], loss_cols[:], start=True, stop=True)
        lsb = small.tile([1, 1], F32, tag="lsb")
        nc.vector.tensor_copy(lsb[:], lps[:])
        nc.sync.dma_start(loss_d, lsb[:])

    split_waits(nc)
    return nc


def prep_inputs(x, enc_W, enc_b, Wqkv, Wo, W1, b1, W2, b2, centers):
    """Host-side marshalling into DMA-friendly layouts (shared across cores)."""
    f32 = np.float32
    shared = {
        "encW": np.ascontiguousarray(enc_W, dtype=f32),
        "encb": np.ascontiguousarray(
            np.broadcast_to(enc_b.astype(f32), (P, D))),
        "wqkv": np.ascontiguousarray(
            Wqkv.astype(f32).reshape(L, DS, P, 3 * D).transpose(0, 2, 1, 3)),
        "wo": np.ascontiguousarray(
            Wo.astype(f32).reshape(L, DS, P, D).transpose(0, 2, 1, 3)),
        "w1": np.ascontiguousarray(
            W1.astype(f32).reshape(L, DS, P, FF).transpose(0, 2, 1, 3)),
        "w2": np.ascontiguousarray(
            W2.astype(f32).reshape(L, FS, P, D).transpose(0, 2, 1, 3)),
        "b1": np.ascontiguousarray(
            b1.astype(f32).reshape(L, FS, P).transpose(0, 2, 1)),
        "b2": np.ascontiguousarray(
            np.broadcast_to(b2.astype(f32)[:, None, :], (L, P, D))),
        "ctrT": np.ascontiguousarray(
            centers.astype(f32).T.reshape(DS, P, K).transpose(1, 0, 2)),
        "cn2": np.ascontiguousarray(np.broadcast_to(
            (centers.astype(f32) ** 2).sum(1), (P, K))),
        "identr": np.eye(P, dtype=f32),
        "ident32": np.eye(P, dtype=f32),
        "ones": np.ones((P, 1), dtype=f32),
    }
    sel = np.zeros((64, P), dtype=f32)
    sel[0, :64] = 1.0
    sel[32, 64:] = 1.0
    shared["sel"] = sel

    in_maps = []
    for c in range(NCORES):
        xi = np.ascontiguousarray(
            x[c * IPC:(c + 1) * IPC].astype(f32).transpose(0, 2, 1))
        in_maps.append({**shared, "xT": xi})
    return in_maps


_NC = None


def kernel(x, enc_W, enc_b, Wqkv, Wo, W1, b1, W2, b2, centers, _want_trace=False):
    global _NC
    if _NC is None:
        _NC = build_bass()
    in_maps = prep_inputs(np.asarray(x), np.asarray(enc_W), np.asarray(enc_b),
                          np.asarray(Wqkv), np.asarray(Wo), np.asarray(W1),
                          np.asarray(b1), np.asarray(W2), np.asarray(b2),
                          np.asarray(centers))
    res = bass_utils.run_bass_kernel_spmd(
        _NC, in_maps, core_ids=list(range(NCORES)), trace=_want_trace)
    outs = [r["out_seq"] for r in res.results]
    loss_parts = [float(r["loss_part"][0, 0]) for r in res.results]
    output_seq = np.concatenate(outs, axis=0)
    loss = np.float32(sum(loss_parts) / (B * D))
    if _want_trace:
        kernel._last = res
    return loss, output_seq
